# revision 18
# baseline (speedup 1.0000x reference)
"""Trainium2 Bass kernel for nn_Attention_Module (submitter/reviewer attention pooling).

Reference math:
    Q  = submitter_emb @ Wq.T + bq                      [B, A]
    K  = einsum('blt,at->bla', reviewer_emb, Wk) + bk   [B, L, A]
    e  = einsum('ba,bla->bl', Q, K) / sqrt(A)           [B, L]
    ww = einsum('bl,blt->bt', e, reviewer_emb)          [B, T]

Algebraic rewrite (exact reassociation):
    qt[b, t] = sum_a Q[b, a] * Wk[a, t]        (= Q @ Wk,   [B, T], tiny)
    cb[b]    = sum_a Q[b, a] * bk[a]
    e[b, l]  = (reviewer[b, l, :] . qt[b, :] + cb[b]) / sqrt(A)
    ww[b, t] = sum_l e[b, l] * reviewer[b, l, t]

One streaming pass over reviewer_emb per core: a fused DVE multiply+reduce
(scalar_tensor_tensor with accum_out) computes raw e per 128-row chunk, a
ScalarE activation adds the (pre-scaled) cb bias, and PE matmuls with e as
the stationary operand accumulate ww.

All large PE matmuls use float32r (single-pass fp32, 1 cycle/row for
>=256-wide moving operands vs 4 for exact fp32; requires operands in
tiles declared float32r and moving free dim >= 2).  Tolerance is 2e-2;
fp32r keeps the result ~2.6e-4.  The prologue runs weights as the
stationary operand so Q lands already transposed (no PE transposes).

An A-sharded prologue + AllReduce was tried and rejected: a minimal 131KB
8-core AllReduce costs ~90us fixed latency on this runtime, far more than
the ~19us of DMA it saves.  Weights are replicated instead.

Sharding: data-parallel over batch B=32 -> 4 batches per core x 8 cores.
Host-side prep is layout-only (transpose/reshape/slice for DMA efficiency);
all input-dependent arithmetic runs on device.
"""

import numpy as np

import concourse.bass as bass
import concourse.bacc as bacc
import concourse.tile as tile
from concourse.tile_rust import add_dep_helper
from concourse import mybir
from concourse.bass_utils import run_bass_kernel_spmd

# Problem shapes (hardcoded per contract)
B, L, T, A = 32, 2048, 1024, 1024
NCORES = 8
BLOC = B // NCORES          # 4 batches per core
P = 128                     # partitions
NCH = T // P                # 8 chunks of 128 along T/A
WQP = 120                   # Wq t'-chunk rows (8-way DMA descriptor split)
NWQ = 8                     # main Wq chunks; tail chunk holds the rest
WQT = T - WQP * NWQ         # 64-row Wq tail chunk
LTILE = 4                   # reviewer rows per partition per DMA tile
ROWS_PER_TILE = P * LTILE   # 512 rows -> 2 MiB per DMA
NTI = L // ROWS_PER_TILE    # 4 DMA tiles per batch
NTILES = BLOC * NTI         # 16 reviewer tiles per core
RTBUFS = 6                  # reviewer stream ring depth (16 KiB/partition)
TP1 = T + 1
SCALE = 1.0 / float(np.sqrt(A))

F32 = mybir.dt.float32
F32R = mybir.dt.float32r
FT = mybir.ActivationFunctionType
OP = mybir.AluOpType


def _build():
    nc = bacc.Bacc("TRN2", target_bir_lowering=False, debug=False, num_devices=NCORES)

    rev = nc.dram_tensor("rev", [BLOC, L, T], F32, kind="ExternalInput").ap()
    # subt[p, j, b] = submitter[4*core + b, 120j + p]   (own batches, t-major)
    subt = nc.dram_tensor("subt", [WQP, NWQ, BLOC], F32, kind="ExternalInput").ap()
    subt8 = nc.dram_tensor("subt8", [WQT, BLOC], F32, kind="ExternalInput").ap()
    # wqtf[p, j, a] = Wq[a, 120j + p]  (Wq.T, t-chunked 8x120; 64-row tail in
    # wqtf8).  120 partitions -> 120 descriptors -> 8-way engine split: the
    # whole Wq load rides DMA engines 0-7, rebalancing the queue against the
    # persistently ~18%-slower engine 15 (which keeps its reviewer share).
    wqtf = nc.dram_tensor("wqtf", [WQP, NWQ, A], F32, kind="ExternalInput").ap()
    wqtf8 = nc.dram_tensor("wqtf8", [WQT, A], F32, kind="ExternalInput").ap()
    # wkf[p, c, t] = Wk[128c + p, t]                    (a-major chunks)
    wkf = nc.dram_tensor("wkf", [P, NCH, T], F32, kind="ExternalInput").ap()
    bqr = nc.dram_tensor("bqr", [1, A], F32, kind="ExternalInput").ap()
    # bk8[p, c] = bk[128c + p]
    bk8 = nc.dram_tensor("bk8", [P, NCH], F32, kind="ExternalInput").ap()
    ones = nc.dram_tensor("ones", [1, BLOC], F32, kind="ExternalInput").ap()
    # sel4[q, 128b + p] = (q == b): row-broadcast selector
    sel4 = nc.dram_tensor("sel4", [BLOC, BLOC * P], F32, kind="ExternalInput").ap()
    out = nc.dram_tensor("out", [BLOC, T], F32, kind="ExternalOutput").ap()

    with tile.TileContext(nc) as tc:
        with (
            tc.tile_pool(name="small", bufs=1) as small,
            tc.tile_pool(name="wq", bufs=1) as wqp,
            tc.tile_pool(name="wk", bufs=1) as wkp,
            tc.tile_pool(name="rp", bufs=1) as rp,
            tc.tile_pool(name="scr", bufs=1) as scrp,
            tc.tile_pool(name="ep", bufs=6) as ep,
            tc.tile_pool(name="wwp", bufs=2) as wwp,
            tc.tile_pool(name="psA", bufs=2, space="PSUM") as psA,
            tc.tile_pool(name="psq", bufs=2, space="PSUM") as psq,
            tc.tile_pool(name="pss", bufs=1, space="PSUM") as pss,
        ):
            # ---- weight loads.  Wq: one [120, 8, A] DMA = 120 descriptors of
            #      32 KiB -> 8-way engine split (engines 0-7, 15 descs each).
            #      Wk: two [128, 4, T] DMAs = 128 descriptors of 16 KiB each,
            #      16-way, same descriptor size as the reviewer stream; the
            #      split also lets the first half of the qt matmul chain
            #      start one DMA earlier. ----
            wqt_sb = wqp.tile([WQP, NWQ, A], F32R, name="wqt_sb", tag="wqt_sb")
            nc.sync.dma_start(out=wqt_sb, in_=wqtf.bitcast(F32R))
            wqt8_sb = wqp.tile([WQT, A], F32R, name="wqt8_sb", tag="wqt8_sb")
            nc.sync.dma_start(out=wqt8_sb, in_=wqtf8.bitcast(F32R))
            wk_sb = wkp.tile([P, NCH, T], F32R, name="wk_sb", tag="wk_sb")
            nc.sync.dma_start(out=wk_sb[:, 0 : NCH // 2, :], in_=wkf.bitcast(F32R)[:, 0 : NCH // 2, :])
            nc.sync.dma_start(out=wk_sb[:, NCH // 2 :, :], in_=wkf.bitcast(F32R)[:, NCH // 2 :, :])

            # ---- small prologue loads on scalar ----
            subt_sb = small.tile([WQP, NWQ, BLOC], F32R, name="subt_sb", tag="subt_sb")
            nc.scalar.dma_start(out=subt_sb, in_=subt.bitcast(F32R))
            subt8_sb = small.tile([WQT, BLOC], F32R, name="subt8_sb", tag="subt8_sb")
            nc.scalar.dma_start(out=subt8_sb, in_=subt8.bitcast(F32R))
            bqr_sb = small.tile([1, A], F32R, name="bqr_sb", tag="bqr_sb")
            nc.scalar.dma_start(out=bqr_sb, in_=bqr.bitcast(F32R))
            bk8_sb = small.tile([P, NCH], F32, name="bk8_sb", tag="bk8_sb")
            nc.scalar.dma_start(out=bk8_sb, in_=bk8)
            ones_sb = small.tile([1, BLOC], F32R, name="ones_sb", tag="ones_sb")
            nc.scalar.dma_start(out=ones_sb, in_=ones.bitcast(F32R))
            sel_sb = small.tile([BLOC, BLOC * P], F32R, name="sel_sb", tag="sel_sb")
            nc.scalar.dma_start(out=sel_sb, in_=sel4.bitcast(F32R))

            # ---- reviewer stream ring (contiguous 16 KiB partition lines) ----
            rtb = [
                rp.tile([P, LTILE, T], F32R, name=f"rtb{k}", tag=f"rtb{k}")
                for k in range(RTBUFS)
            ]
            rt_dmas = []

            def _issue_rt(n):
                # Whole 2 MiB tiles (16 KiB descriptor lines, ~92% DMA
                # efficiency), max 4 in flight via the stagger chain so
                # completions stay roughly ordered.  The LAST tile is split
                # into 4 chunk DMAs (each still 128 descriptors, 16-way) so
                # the end-of-stream serial compute is one chunk deep.
                b, ti = divmod(n, NTI)
                k = n % RTBUFS
                dep = rt_dmas[-4] if len(rt_dmas) >= 4 else None
                if n < NTILES - 1:
                    d = nc.sync.dma_start(
                        out=rtb[k],
                        in_=rev.bitcast(F32R)[
                            b, ti * ROWS_PER_TILE : (ti + 1) * ROWS_PER_TILE, :
                        ].rearrange("(p f) t -> p f t", f=LTILE),
                    )
                    if dep is not None:
                        add_dep_helper(d.ins, dep.ins, reason="stagger rt")
                    rt_dmas.append(d)
                else:
                    # chunk i holds rows [r0 + 128i, r0 + 128(i+1)): plain
                    # contiguous [128, T] APs; e/ww math is row-order-agnostic
                    r0 = ti * ROWS_PER_TILE
                    for i in range(LTILE):
                        d = nc.sync.dma_start(
                            out=rtb[k][:, i, :],
                            in_=rev.bitcast(F32R)[b, r0 + i * P : r0 + (i + 1) * P, :],
                        )
                        if dep is not None:
                            add_dep_helper(d.ins, dep.ins, reason="stagger rt")
                        rt_dmas.append(d)

            for n in range(RTBUFS):
                _issue_rt(n)

            # ---- QT[a, b] = sum_t sub[b, t] Wq[a, t] + bq[a], transposed
            #      on the fly: Wq.T chunks stationary, submitter moving;
            #      t' contraction runs 8x120 + 64 to match the Wq layout ----
            psQT = pss.tile([P, NCH * BLOC], F32, name="psQT", tag="pss")
            for c in range(NCH):
                o = psQT[:, c * BLOC : (c + 1) * BLOC]
                for j in range(NWQ):
                    nc.tensor.matmul(
                        o,
                        wqt_sb[:, j, c * P : (c + 1) * P],
                        subt_sb[:, j, :],
                        start=(j == 0),
                        stop=False,
                    )
                nc.tensor.matmul(
                    o,
                    wqt8_sb[:, c * P : (c + 1) * P],
                    subt8_sb,
                    start=False,
                    stop=False,
                )
                nc.tensor.matmul(
                    o,
                    bqr_sb[:, c * P : (c + 1) * P],
                    ones_sb,
                    start=False,
                    stop=True,
                )
            QT_sb = small.tile([P, NCH, BLOC], F32R, name="QT_sb", tag="QT_sb")
            nc.scalar.copy(QT_sb.rearrange("p c b -> p (c b)"), psQT)

            # ---- qt[b, t] = Q @ Wk (+ cb), scaled by 1/sqrt(A) ----
            psq2 = psA.tile([BLOC, T], F32, name="psq2", tag="psA")
            for h in range(2):
                o = psq2[:, h * 512 : (h + 1) * 512]
                for c in range(NCH):
                    nc.tensor.matmul(
                        o,
                        QT_sb[:, c, :],
                        wk_sb[:, c, h * 512 : (h + 1) * 512],
                        start=(c == 0),
                        stop=(c == NCH - 1),
                    )
            cb_ps = pss.tile([BLOC, 1], F32, name="cb_ps", tag="pss")
            for c in range(NCH):
                nc.tensor.matmul(
                    cb_ps,
                    QT_sb[:, c, :].bitcast(F32),
                    bk8_sb[:, c : c + 1],
                    start=(c == 0),
                    stop=(c == NCH - 1),
                )
            qt4_sb = small.tile([BLOC, TP1], F32R, name="qt4_sb", tag="qt4_sb")
            for h in range(2):
                nc.scalar.mul(
                    qt4_sb[:, h * 512 : (h + 1) * 512],
                    psq2[:, h * 512 : (h + 1) * 512],
                    SCALE,
                )
            nc.scalar.mul(qt4_sb[:, T:TP1], cb_ps, SCALE)

            # ---- broadcast a batch's qt row to 128 partitions on PE;
            #      col T carries the scaled cb bias.  Emitted lazily: qb[0]
            #      before the stream, qb[b+1] during batch b's stream, so
            #      the PE<->scalar sem ping-pong (~3 us/batch) overlaps the
            #      stream instead of delaying the first DVE chunk ----
            qb_all = small.tile([P, BLOC, TP1], F32, name="qb_all", tag="qb_all")

            def _emit_qb(b):
                for h in range(2):
                    qps = psq.tile([P, 512], F32, name="qps", tag="psq")
                    nc.tensor.matmul(
                        qps,
                        sel_sb[:, b * P : (b + 1) * P],
                        qt4_sb[:, h * 512 : (h + 1) * 512],
                        start=True,
                        stop=True,
                    )
                    nc.scalar.copy(qb_all[:, b, h * 512 : (h + 1) * 512], qps)
                qpc = pss.tile([P, 1], F32, name="qpc", tag="pss")
                nc.tensor.matmul(
                    qpc,
                    sel_sb[:, b * P : (b + 1) * P].bitcast(F32),
                    qt4_sb[:, T:TP1].bitcast(F32),
                    start=True,
                    stop=True,
                )
                nc.scalar.copy(qb_all[:, b, T:TP1], qpc)

            qb_t = [qb_all[:, b, :] for b in range(BLOC)]
            _emit_qb(0)

            # ---- main stream: e = (rt . qt)+cb ; ww += e.T-weighted rows ----
            for b in range(BLOC):
                ps_ww = psA.tile([1, T], F32, name="ps_ww", tag="psA")
                for ti in range(NTI):
                    if ti == 1 and b + 1 < BLOC:
                        _emit_qb(b + 1)
                    n = b * NTI + ti
                    k = n % RTBUFS
                    e_raw = ep.tile([P, LTILE], F32, name="e_raw", tag="e_raw")
                    e_t = ep.tile([P, LTILE], F32R, name="e_t", tag="e_t")
                    for i in range(LTILE):
                        scr = scrp.tile([P, T], F32, name="scr", tag="scr")
                        nc.vector.scalar_tensor_tensor(
                            out=scr,
                            in0=rtb[k][:, i, :].bitcast(F32),
                            scalar=1.0,
                            in1=qb_t[b][:, 0:T],
                            op0=OP.bypass,
                            op1=OP.mult,
                            accum_out=e_raw[:, i : i + 1],
                        )
                        nc.scalar.activation(
                            e_t[:, i : i + 1],
                            e_raw[:, i : i + 1],
                            FT.Identity,
                            bias=qb_t[b][:, T : T + 1],
                        )
                        for h in range(2):
                            nc.tensor.matmul(
                                ps_ww[0:1, h * 512 : (h + 1) * 512],
                                e_t[:, i : i + 1],
                                rtb[k][:, i, h * 512 : (h + 1) * 512],
                                start=(ti == 0 and i == 0),
                                stop=(ti == NTI - 1 and i == LTILE - 1),
                            )
                    if n + RTBUFS < NTILES:
                        _issue_rt(n + RTBUFS)
                ww_sb = wwp.tile([1, T], F32, name="ww_sb", tag="ww_sb")
                nc.scalar.copy(ww_sb[0:1, 0:512], ps_ww[0:1, 0:512])
                nc.scalar.copy(ww_sb[0:1, 512:1024], ps_ww[0:1, 512:1024])
                nc.scalar.dma_start(out=out[b : b + 1, :], in_=ww_sb)

    nc.compile()
    return nc


_NC = None


def _get_nc():
    global _NC
    if _NC is None:
        _NC = _build()
    return _NC


def _in_maps(submitter_emb, reviewer_emb, Wq, bq, Wk, bk):
    submitter_emb = np.ascontiguousarray(submitter_emb, dtype=np.float32)
    reviewer_emb = np.ascontiguousarray(reviewer_emb, dtype=np.float32)
    Wq = np.asarray(Wq, dtype=np.float32)
    Wk = np.asarray(Wk, dtype=np.float32)
    bq = np.asarray(bq, dtype=np.float32)
    bk = np.asarray(bk, dtype=np.float32)

    # wqtf[p, j, a] = Wq[a, 120j + p]; wqtf8[p, a] = Wq[a, 960 + p]
    wqt = np.ascontiguousarray(Wq.T)
    wqtf = np.ascontiguousarray(
        wqt[: WQP * NWQ].reshape(NWQ, WQP, A).transpose(1, 0, 2)
    )
    wqtf8 = np.ascontiguousarray(wqt[WQP * NWQ :])
    # wkf[p, c, t] = Wk[128c + p, t]
    wkf = np.ascontiguousarray(Wk.reshape(NCH, P, T).transpose(1, 0, 2))
    bqr = np.ascontiguousarray(bq.reshape(1, A))
    bk8 = np.ascontiguousarray(bk.reshape(NCH, P).T)
    ones = np.ones((1, BLOC), dtype=np.float32)
    sel4 = np.zeros((BLOC, BLOC * P), dtype=np.float32)
    for b_ in range(BLOC):
        sel4[b_, b_ * P : (b_ + 1) * P] = 1.0

    in_maps = []
    for core in range(NCORES):
        lo, hi = core * BLOC, (core + 1) * BLOC
        st = np.ascontiguousarray(submitter_emb[lo:hi].T)
        subt = np.ascontiguousarray(
            st[: WQP * NWQ].reshape(NWQ, WQP, BLOC).transpose(1, 0, 2)
        )
        subt8 = np.ascontiguousarray(st[WQP * NWQ :])
        in_maps.append(
            {
                "rev": reviewer_emb[lo:hi],
                "subt": subt,
                "subt8": subt8,
                "wqtf": wqtf,
                "wqtf8": wqtf8,
                "wkf": wkf,
                "bqr": bqr,
                "bk8": bk8,
                "ones": ones,
                "sel4": sel4,
            }
        )
    return in_maps


def kernel(
    submitter_emb: np.ndarray,
    reviewer_emb: np.ndarray,
    Wq: np.ndarray,
    bq: np.ndarray,
    Wk: np.ndarray,
    bk: np.ndarray,
) -> np.ndarray:
    nc = _get_nc()
    in_maps = _in_maps(submitter_emb, reviewer_emb, Wq, bq, Wk, bk)
    res = run_bass_kernel_spmd(nc, in_maps, core_ids=list(range(NCORES)))
    return np.concatenate([res.results[c]["out"] for c in range(NCORES)], axis=0)



# revision 19
# speedup vs baseline: 1.0221x; 1.0221x over previous
"""Trainium2 Bass kernel for nn_Attention_Module (submitter/reviewer attention pooling).

Reference math:
    Q  = submitter_emb @ Wq.T + bq                      [B, A]
    K  = einsum('blt,at->bla', reviewer_emb, Wk) + bk   [B, L, A]
    e  = einsum('ba,bla->bl', Q, K) / sqrt(A)           [B, L]
    ww = einsum('bl,blt->bt', e, reviewer_emb)          [B, T]

Algebraic rewrite (exact reassociation):
    qt[b, t] = sum_a Q[b, a] * Wk[a, t]        (= Q @ Wk,   [B, T], tiny)
    cb[b]    = sum_a Q[b, a] * bk[a]
    e[b, l]  = (reviewer[b, l, :] . qt[b, :] + cb[b]) / sqrt(A)
    ww[b, t] = sum_l e[b, l] * reviewer[b, l, t]

One streaming pass over reviewer_emb per core: a fused DVE multiply+reduce
(scalar_tensor_tensor with accum_out) computes raw e per 128-row chunk, a
ScalarE activation adds the (pre-scaled) cb bias, and PE matmuls with e as
the stationary operand accumulate ww.

All large PE matmuls use float32r (single-pass fp32, 1 cycle/row for
>=256-wide moving operands vs 4 for exact fp32; requires operands in
tiles declared float32r and moving free dim >= 2).  Tolerance is 2e-2;
fp32r keeps the result ~2.6e-4.  The prologue runs weights as the
stationary operand so Q lands already transposed (no PE transposes).

An A-sharded prologue + AllReduce was tried and rejected: a minimal 131KB
8-core AllReduce costs ~90us fixed latency on this runtime, far more than
the ~19us of DMA it saves.  Weights are replicated instead.

Sharding: data-parallel over batch B=32 -> 4 batches per core x 8 cores.
Host-side prep is layout-only (transpose/reshape/slice for DMA efficiency);
all input-dependent arithmetic runs on device.
"""

import numpy as np

import concourse.bass as bass
import concourse.bacc as bacc
import concourse.tile as tile
from concourse.tile_rust import add_dep_helper
from concourse import mybir
from concourse.bass_utils import run_bass_kernel_spmd

# Problem shapes (hardcoded per contract)
B, L, T, A = 32, 2048, 1024, 1024
NCORES = 8
BLOC = B // NCORES          # 4 batches per core
P = 128                     # partitions
NCH = T // P                # 8 chunks of 128 along T/A
LTILE = 4                   # reviewer rows per partition per DMA tile
ROWS_PER_TILE = P * LTILE   # 512 rows -> 2 MiB per DMA
NTI = L // ROWS_PER_TILE    # 4 DMA tiles per batch
NTILES = BLOC * NTI         # 16 reviewer tiles per core
RTBUFS = 6                  # reviewer stream ring depth (16 KiB/partition)
TP1 = T + 1
SCALE = 1.0 / float(np.sqrt(A))

F32 = mybir.dt.float32
F32R = mybir.dt.float32r
FT = mybir.ActivationFunctionType
OP = mybir.AluOpType


def _build():
    nc = bacc.Bacc("TRN2", target_bir_lowering=False, debug=False, num_devices=NCORES)

    rev = nc.dram_tensor("rev", [BLOC, L, T], F32, kind="ExternalInput").ap()
    # subt[p, j, b] = submitter[4*core + b, 128j + p]   (own batches, t-major)
    subt = nc.dram_tensor("subt", [P, NCH, BLOC], F32, kind="ExternalInput").ap()
    # wqtf[p, j, a] = Wq[a, 128j + p]                   (Wq.T, t-major chunks)
    wqtf = nc.dram_tensor("wqtf", [P, NCH, A], F32, kind="ExternalInput").ap()
    # wkf[p, c, t] = Wk[128c + p, t]                    (a-major chunks)
    wkf = nc.dram_tensor("wkf", [P, NCH, T], F32, kind="ExternalInput").ap()
    bqr = nc.dram_tensor("bqr", [1, A], F32, kind="ExternalInput").ap()
    # bk8[p, c] = bk[128c + p]
    bk8 = nc.dram_tensor("bk8", [P, NCH], F32, kind="ExternalInput").ap()
    ones = nc.dram_tensor("ones", [1, BLOC], F32, kind="ExternalInput").ap()
    # sel4[q, 128b + p] = (q == b): row-broadcast selector
    sel4 = nc.dram_tensor("sel4", [BLOC, BLOC * P], F32, kind="ExternalInput").ap()
    out = nc.dram_tensor("out", [BLOC, T], F32, kind="ExternalOutput").ap()

    with tile.TileContext(nc) as tc:
        with (
            tc.tile_pool(name="small", bufs=1) as small,
            tc.tile_pool(name="wq", bufs=1) as wqp,
            tc.tile_pool(name="wk", bufs=1) as wkp,
            tc.tile_pool(name="rp", bufs=1) as rp,
            tc.tile_pool(name="scr", bufs=1) as scrp,
            tc.tile_pool(name="ep", bufs=6) as ep,
            tc.tile_pool(name="wwp", bufs=2) as wwp,
            tc.tile_pool(name="psA", bufs=2, space="PSUM") as psA,
            tc.tile_pool(name="psq", bufs=2, space="PSUM") as psq,
            tc.tile_pool(name="pss", bufs=1, space="PSUM") as pss,
        ):
            # ---- weight loads: ONE DMA per matrix -> 128 descriptors of a
            #      contiguous 32 KiB partition line each, uniform 16-way
            #      engine split (8 x 32 KiB per engine). ----
            wqt_sb = wqp.tile([P, NCH, A], F32R, name="wqt_sb", tag="wqt_sb")
            nc.sync.dma_start(out=wqt_sb, in_=wqtf.bitcast(F32R))
            wk_sb = wkp.tile([P, NCH, T], F32R, name="wk_sb", tag="wk_sb")
            nc.sync.dma_start(out=wk_sb, in_=wkf.bitcast(F32R))

            # ---- small prologue loads on scalar ----
            subt_sb = small.tile([P, NCH, BLOC], F32R, name="subt_sb", tag="subt_sb")
            nc.scalar.dma_start(out=subt_sb, in_=subt.bitcast(F32R))
            bqr_sb = small.tile([1, A], F32R, name="bqr_sb", tag="bqr_sb")
            nc.scalar.dma_start(out=bqr_sb, in_=bqr.bitcast(F32R))
            bk8_sb = small.tile([P, NCH], F32, name="bk8_sb", tag="bk8_sb")
            nc.scalar.dma_start(out=bk8_sb, in_=bk8)
            ones_sb = small.tile([1, BLOC], F32R, name="ones_sb", tag="ones_sb")
            nc.scalar.dma_start(out=ones_sb, in_=ones.bitcast(F32R))
            sel_sb = small.tile([BLOC, BLOC * P], F32R, name="sel_sb", tag="sel_sb")
            nc.scalar.dma_start(out=sel_sb, in_=sel4.bitcast(F32R))

            # ---- reviewer stream ring (contiguous 16 KiB partition lines) ----
            rtb = [
                rp.tile([P, LTILE, T], F32R, name=f"rtb{k}", tag=f"rtb{k}")
                for k in range(RTBUFS)
            ]
            rt_dmas = []

            def _issue_rt(n):
                # Whole 2 MiB tiles (16 KiB descriptor lines, ~92% DMA
                # efficiency), max 4 in flight via the stagger chain so
                # completions stay roughly ordered.  The LAST tile is split
                # into 4 chunk DMAs (each still 128 descriptors, 16-way) so
                # the end-of-stream serial compute is one chunk deep.
                b, ti = divmod(n, NTI)
                k = n % RTBUFS
                dep = rt_dmas[-4] if len(rt_dmas) >= 4 else None
                if n < NTILES - 1:
                    d = nc.sync.dma_start(
                        out=rtb[k],
                        in_=rev.bitcast(F32R)[
                            b, ti * ROWS_PER_TILE : (ti + 1) * ROWS_PER_TILE, :
                        ].rearrange("(p f) t -> p f t", f=LTILE),
                    )
                    if dep is not None:
                        add_dep_helper(d.ins, dep.ins, reason="stagger rt")
                    rt_dmas.append(d)
                else:
                    # chunk i holds rows [r0 + 128i, r0 + 128(i+1)): plain
                    # contiguous [128, T] APs; e/ww math is row-order-agnostic
                    r0 = ti * ROWS_PER_TILE
                    for i in range(LTILE):
                        d = nc.sync.dma_start(
                            out=rtb[k][:, i, :],
                            in_=rev.bitcast(F32R)[b, r0 + i * P : r0 + (i + 1) * P, :],
                        )
                        if dep is not None:
                            add_dep_helper(d.ins, dep.ins, reason="stagger rt")
                        rt_dmas.append(d)

            for n in range(RTBUFS):
                _issue_rt(n)

            # ---- QT[a, b] = sum_t sub[b, t] Wq[a, t] + bq[a], transposed
            #      on the fly: Wq.T chunks stationary, submitter moving;
            psQT = pss.tile([P, NCH * BLOC], F32, name="psQT", tag="pss")
            for c in range(NCH):
                o = psQT[:, c * BLOC : (c + 1) * BLOC]
                for j in range(NCH):
                    nc.tensor.matmul(
                        o,
                        wqt_sb[:, j, c * P : (c + 1) * P],
                        subt_sb[:, j, :],
                        start=(j == 0),
                        stop=False,
                    )
                nc.tensor.matmul(
                    o,
                    bqr_sb[:, c * P : (c + 1) * P],
                    ones_sb,
                    start=False,
                    stop=True,
                )
            QT_sb = small.tile([P, NCH, BLOC], F32R, name="QT_sb", tag="QT_sb")
            nc.scalar.copy(QT_sb.rearrange("p c b -> p (c b)"), psQT)

            # ---- qt[b, t] = Q @ Wk (+ cb), scaled by 1/sqrt(A) ----
            psq2 = psA.tile([BLOC, T], F32, name="psq2", tag="psA")
            for h in range(2):
                o = psq2[:, h * 512 : (h + 1) * 512]
                for c in range(NCH):
                    nc.tensor.matmul(
                        o,
                        QT_sb[:, c, :],
                        wk_sb[:, c, h * 512 : (h + 1) * 512],
                        start=(c == 0),
                        stop=(c == NCH - 1),
                    )
            cb_ps = pss.tile([BLOC, 1], F32, name="cb_ps", tag="pss")
            for c in range(NCH):
                nc.tensor.matmul(
                    cb_ps,
                    QT_sb[:, c, :].bitcast(F32),
                    bk8_sb[:, c : c + 1],
                    start=(c == 0),
                    stop=(c == NCH - 1),
                )
            qt4_sb = small.tile([BLOC, TP1], F32R, name="qt4_sb", tag="qt4_sb")
            for h in range(2):
                nc.scalar.mul(
                    qt4_sb[:, h * 512 : (h + 1) * 512],
                    psq2[:, h * 512 : (h + 1) * 512],
                    SCALE,
                )
            nc.scalar.mul(qt4_sb[:, T:TP1], cb_ps, SCALE)

            # ---- broadcast a batch's qt row to 128 partitions on PE;
            #      col T carries the scaled cb bias.  Emitted lazily: qb[0]
            #      before the stream, qb[b+1] during batch b's stream, so
            #      the PE<->scalar sem ping-pong (~3 us/batch) overlaps the
            #      stream instead of delaying the first DVE chunk ----
            qb_all = small.tile([P, BLOC, TP1], F32, name="qb_all", tag="qb_all")

            def _emit_qb(b):
                for h in range(2):
                    qps = psq.tile([P, 512], F32, name="qps", tag="psq")
                    nc.tensor.matmul(
                        qps,
                        sel_sb[:, b * P : (b + 1) * P],
                        qt4_sb[:, h * 512 : (h + 1) * 512],
                        start=True,
                        stop=True,
                    )
                    nc.scalar.copy(qb_all[:, b, h * 512 : (h + 1) * 512], qps)
                qpc = pss.tile([P, 1], F32, name="qpc", tag="pss")
                nc.tensor.matmul(
                    qpc,
                    sel_sb[:, b * P : (b + 1) * P].bitcast(F32),
                    qt4_sb[:, T:TP1].bitcast(F32),
                    start=True,
                    stop=True,
                )
                nc.scalar.copy(qb_all[:, b, T:TP1], qpc)

            qb_t = [qb_all[:, b, :] for b in range(BLOC)]
            _emit_qb(0)

            # ---- main stream: e = (rt . qt)+cb ; ww += e.T-weighted rows ----
            for b in range(BLOC):
                ps_ww = psA.tile([1, T], F32, name="ps_ww", tag="psA")
                for ti in range(NTI):
                    if ti == 1 and b + 1 < BLOC:
                        _emit_qb(b + 1)
                    n = b * NTI + ti
                    k = n % RTBUFS
                    e_raw = ep.tile([P, LTILE], F32, name="e_raw", tag="e_raw")
                    e_t = ep.tile([P, LTILE], F32R, name="e_t", tag="e_t")
                    for i in range(LTILE):
                        scr = scrp.tile([P, T], F32, name="scr", tag="scr")
                        nc.vector.scalar_tensor_tensor(
                            out=scr,
                            in0=rtb[k][:, i, :].bitcast(F32),
                            scalar=1.0,
                            in1=qb_t[b][:, 0:T],
                            op0=OP.bypass,
                            op1=OP.mult,
                            accum_out=e_raw[:, i : i + 1],
                        )
                        nc.scalar.activation(
                            e_t[:, i : i + 1],
                            e_raw[:, i : i + 1],
                            FT.Identity,
                            bias=qb_t[b][:, T : T + 1],
                        )
                        for h in range(2):
                            nc.tensor.matmul(
                                ps_ww[0:1, h * 512 : (h + 1) * 512],
                                e_t[:, i : i + 1],
                                rtb[k][:, i, h * 512 : (h + 1) * 512],
                                start=(ti == 0 and i == 0),
                                stop=(ti == NTI - 1 and i == LTILE - 1),
                            )
                    if n + RTBUFS < NTILES:
                        _issue_rt(n + RTBUFS)
                ww_sb = wwp.tile([1, T], F32, name="ww_sb", tag="ww_sb")
                nc.scalar.copy(ww_sb[0:1, 0:512], ps_ww[0:1, 0:512])
                nc.scalar.copy(ww_sb[0:1, 512:1024], ps_ww[0:1, 512:1024])
                nc.scalar.dma_start(out=out[b : b + 1, :], in_=ww_sb)

    nc.compile()
    return nc


_NC = None


def _get_nc():
    global _NC
    if _NC is None:
        _NC = _build()
    return _NC


def _in_maps(submitter_emb, reviewer_emb, Wq, bq, Wk, bk):
    submitter_emb = np.ascontiguousarray(submitter_emb, dtype=np.float32)
    reviewer_emb = np.ascontiguousarray(reviewer_emb, dtype=np.float32)
    Wq = np.asarray(Wq, dtype=np.float32)
    Wk = np.asarray(Wk, dtype=np.float32)
    bq = np.asarray(bq, dtype=np.float32)
    bk = np.asarray(bk, dtype=np.float32)

    # wqtf[p, j, a] = Wq[a, 128j + p]
    wqtf = np.ascontiguousarray(Wq.T.reshape(NCH, P, A).transpose(1, 0, 2))
    # wkf[p, c, t] = Wk[128c + p, t]
    wkf = np.ascontiguousarray(Wk.reshape(NCH, P, T).transpose(1, 0, 2))
    bqr = np.ascontiguousarray(bq.reshape(1, A))
    bk8 = np.ascontiguousarray(bk.reshape(NCH, P).T)
    ones = np.ones((1, BLOC), dtype=np.float32)
    sel4 = np.zeros((BLOC, BLOC * P), dtype=np.float32)
    for b_ in range(BLOC):
        sel4[b_, b_ * P : (b_ + 1) * P] = 1.0

    in_maps = []
    for core in range(NCORES):
        lo, hi = core * BLOC, (core + 1) * BLOC
        subt = np.ascontiguousarray(
            submitter_emb[lo:hi].T.reshape(NCH, P, BLOC).transpose(1, 0, 2)
        )
        in_maps.append(
            {
                "rev": reviewer_emb[lo:hi],
                "subt": subt,
                "wqtf": wqtf,
                "wkf": wkf,
                "bqr": bqr,
                "bk8": bk8,
                "ones": ones,
                "sel4": sel4,
            }
        )
    return in_maps


def kernel(
    submitter_emb: np.ndarray,
    reviewer_emb: np.ndarray,
    Wq: np.ndarray,
    bq: np.ndarray,
    Wk: np.ndarray,
    bk: np.ndarray,
) -> np.ndarray:
    nc = _get_nc()
    in_maps = _in_maps(submitter_emb, reviewer_emb, Wq, bq, Wk, bk)
    res = run_bass_kernel_spmd(nc, in_maps, core_ids=list(range(NCORES)))
    return np.concatenate([res.results[c]["out"] for c in range(NCORES)], axis=0)



# revision 20
# speedup vs baseline: 1.0654x; 1.0424x over previous
"""Trainium2 Bass kernel for nn_Attention_Module (submitter/reviewer attention pooling).

Reference math:
    Q  = submitter_emb @ Wq.T + bq                      [B, A]
    K  = einsum('blt,at->bla', reviewer_emb, Wk) + bk   [B, L, A]
    e  = einsum('ba,bla->bl', Q, K) / sqrt(A)           [B, L]
    ww = einsum('bl,blt->bt', e, reviewer_emb)          [B, T]

Algebraic rewrite (exact reassociation):
    qt[b, t] = sum_a Q[b, a] * Wk[a, t]        (= Q @ Wk,   [B, T], tiny)
    cb[b]    = sum_a Q[b, a] * bk[a]
    e[b, l]  = (reviewer[b, l, :] . qt[b, :] + cb[b]) / sqrt(A)
    ww[b, t] = sum_l e[b, l] * reviewer[b, l, t]

One streaming pass over reviewer_emb per core: a fused DVE multiply+reduce
(scalar_tensor_tensor with accum_out) computes raw e per 128-row chunk, a
ScalarE activation adds the (pre-scaled) cb bias, and PE matmuls with e as
the stationary operand accumulate ww.

All large PE matmuls use float32r (single-pass fp32, 1 cycle/row for
>=256-wide moving operands vs 4 for exact fp32; requires operands in
tiles declared float32r and moving free dim >= 2).  Tolerance is 2e-2;
fp32r keeps the result ~2.6e-4.  The prologue runs weights as the
stationary operand so Q lands already transposed (no PE transposes).

An A-sharded prologue + AllReduce was tried and rejected: a minimal 131KB
8-core AllReduce costs ~90us fixed latency on this runtime, far more than
the ~19us of DMA it saves.  Weights are replicated instead.

Sharding: data-parallel over batch B=32 -> 4 batches per core x 8 cores.
Host-side prep is layout-only (transpose/reshape/slice for DMA efficiency);
all input-dependent arithmetic runs on device.
"""

import numpy as np

import concourse.bass as bass
import concourse.bacc as bacc
import concourse.tile as tile
from concourse.tile_rust import add_dep_helper
from concourse import mybir
from concourse.bass_utils import run_bass_kernel_spmd

# Problem shapes (hardcoded per contract)
B, L, T, A = 32, 2048, 1024, 1024
NCORES = 8
BLOC = B // NCORES          # 4 batches per core
P = 128                     # partitions
NCH = T // P                # 8 chunks of 128 along T/A
LTILE = 4                   # reviewer rows per partition per DMA tile
ROWS_PER_TILE = P * LTILE   # 512 rows -> 2 MiB per DMA
NTI = L // ROWS_PER_TILE    # 4 DMA tiles per batch
NTILES = BLOC * NTI         # 16 reviewer tiles per core
RTBUFS = 6                  # reviewer stream ring depth (16 KiB/partition)
TP1 = T + 1
SCALE = 1.0 / float(np.sqrt(A))

F32 = mybir.dt.float32
F32R = mybir.dt.float32r
BF16 = mybir.dt.bfloat16
FT = mybir.ActivationFunctionType
OP = mybir.AluOpType


def _build():
    nc = bacc.Bacc("TRN2", target_bir_lowering=False, debug=False, num_devices=NCORES)

    rev = nc.dram_tensor("rev", [BLOC, L, T], F32, kind="ExternalInput").ap()
    # subt[p, j, b] = submitter[4*core + b, 128j + p]   (own batches, t-major)
    subt = nc.dram_tensor("subt", [P, NCH, BLOC], F32, kind="ExternalInput").ap()
    # wqtf[p, j, a] = Wq[a, 128j + p]                   (Wq.T, t-major chunks)
    wqtf = nc.dram_tensor("wqtf", [P, NCH, A], F32, kind="ExternalInput").ap()
    # wkf[p, c, t] = Wk[128c + p, t]                    (a-major chunks)
    wkf = nc.dram_tensor("wkf", [P, NCH, T], F32, kind="ExternalInput").ap()
    bqr = nc.dram_tensor("bqr", [1, A], F32, kind="ExternalInput").ap()
    # bk8[p, c] = bk[128c + p]
    bk8 = nc.dram_tensor("bk8", [P, NCH], F32, kind="ExternalInput").ap()
    ones = nc.dram_tensor("ones", [1, BLOC], F32, kind="ExternalInput").ap()
    # sel4[q, 128b + p] = (q == b): row-broadcast selector
    sel4 = nc.dram_tensor("sel4", [BLOC, BLOC * P], F32, kind="ExternalInput").ap()
    out = nc.dram_tensor("out", [BLOC, T], F32, kind="ExternalOutput").ap()

    with tile.TileContext(nc) as tc:
        with (
            tc.tile_pool(name="small", bufs=1) as small,
            tc.tile_pool(name="wq", bufs=1) as wqp,
            tc.tile_pool(name="wk", bufs=1) as wkp,
            tc.tile_pool(name="rp", bufs=1) as rp,
            tc.tile_pool(name="scr", bufs=1) as scrp,
            tc.tile_pool(name="ep", bufs=6) as ep,
            tc.tile_pool(name="wwp", bufs=2) as wwp,
            tc.tile_pool(name="psA", bufs=2, space="PSUM") as psA,
            tc.tile_pool(name="psq", bufs=2, space="PSUM") as psq,
            tc.tile_pool(name="pss", bufs=1, space="PSUM") as pss,
        ):
            # ---- weight loads: ONE DMA per matrix -> 128 descriptors of a
            #      contiguous 32 KiB partition line each, uniform 16-way
            #      engine split (8 x 32 KiB per engine). ----
            wqt_sb = wqp.tile([P, NCH, A], F32R, name="wqt_sb", tag="wqt_sb")
            nc.gpsimd.dma_start(out=wqt_sb, in_=wqtf.bitcast(F32R))
            wk_sb = wkp.tile([P, NCH, T], F32R, name="wk_sb", tag="wk_sb")
            nc.gpsimd.dma_start(out=wk_sb, in_=wkf.bitcast(F32R))

            # ---- small prologue loads on scalar ----
            subt_sb = small.tile([P, NCH, BLOC], F32R, name="subt_sb", tag="subt_sb")
            nc.scalar.dma_start(out=subt_sb, in_=subt.bitcast(F32R))
            bqr_sb = small.tile([1, A], F32R, name="bqr_sb", tag="bqr_sb")
            nc.scalar.dma_start(out=bqr_sb, in_=bqr.bitcast(F32R))
            bk8_sb = small.tile([P, NCH], F32, name="bk8_sb", tag="bk8_sb")
            nc.scalar.dma_start(out=bk8_sb, in_=bk8)
            ones_sb = small.tile([1, BLOC], F32R, name="ones_sb", tag="ones_sb")
            nc.scalar.dma_start(out=ones_sb, in_=ones.bitcast(F32R))
            sel_sb = small.tile([BLOC, BLOC * P], F32R, name="sel_sb", tag="sel_sb")
            nc.scalar.dma_start(out=sel_sb, in_=sel4.bitcast(F32R))

            # ---- reviewer stream ring (contiguous 16 KiB partition lines) ----
            rtb = [
                rp.tile([P, LTILE, T], BF16, name=f"rtb{k}", tag=f"rtb{k}")
                for k in range(RTBUFS)
            ]
            rt_dmas = []

            def _issue_rt(n):
                # Whole 2 MiB tiles (16 KiB descriptor lines, ~92% DMA
                # efficiency), max 4 in flight via the stagger chain so
                # completions stay roughly ordered.  The LAST tile is split
                # into 4 chunk DMAs (each still 128 descriptors, 16-way) so
                # the end-of-stream serial compute is one chunk deep.
                b, ti = divmod(n, NTI)
                k = n % RTBUFS
                dep = rt_dmas[-4] if len(rt_dmas) >= 4 else None
                if n < NTILES - 1:
                    d = nc.gpsimd.dma_start(
                        out=rtb[k],
                        in_=rev[
                            b, ti * ROWS_PER_TILE : (ti + 1) * ROWS_PER_TILE, :
                        ].rearrange("(p f) t -> p f t", f=LTILE),
                    )
                    if dep is not None:
                        add_dep_helper(d.ins, dep.ins, reason="stagger rt")
                    rt_dmas.append(d)
                else:
                    # chunk i holds rows [r0 + 128i, r0 + 128(i+1)): plain
                    # contiguous [128, T] APs; e/ww math is row-order-agnostic
                    r0 = ti * ROWS_PER_TILE
                    for i in range(LTILE):
                        d = nc.gpsimd.dma_start(
                            out=rtb[k][:, i, :],
                            in_=rev[b, r0 + i * P : r0 + (i + 1) * P, :],
                        )
                        if dep is not None:
                            add_dep_helper(d.ins, dep.ins, reason="stagger rt")
                        rt_dmas.append(d)

            for n in range(RTBUFS):
                _issue_rt(n)

            # ---- QT[a, b] = sum_t sub[b, t] Wq[a, t] + bq[a], transposed
            #      on the fly: Wq.T chunks stationary, submitter moving;
            psQT = pss.tile([P, NCH * BLOC], F32, name="psQT", tag="pss")
            for c in range(NCH):
                o = psQT[:, c * BLOC : (c + 1) * BLOC]
                for j in range(NCH):
                    nc.tensor.matmul(
                        o,
                        wqt_sb[:, j, c * P : (c + 1) * P],
                        subt_sb[:, j, :],
                        start=(j == 0),
                        stop=False,
                    )
                nc.tensor.matmul(
                    o,
                    bqr_sb[:, c * P : (c + 1) * P],
                    ones_sb,
                    start=False,
                    stop=True,
                )
            QT_sb = small.tile([P, NCH, BLOC], F32R, name="QT_sb", tag="QT_sb")
            nc.scalar.copy(QT_sb.rearrange("p c b -> p (c b)"), psQT)

            # ---- qt[b, t] = Q @ Wk (+ cb), scaled by 1/sqrt(A) ----
            psq2 = psA.tile([BLOC, T], F32, name="psq2", tag="psA")
            for h in range(2):
                o = psq2[:, h * 512 : (h + 1) * 512]
                for c in range(NCH):
                    nc.tensor.matmul(
                        o,
                        QT_sb[:, c, :],
                        wk_sb[:, c, h * 512 : (h + 1) * 512],
                        start=(c == 0),
                        stop=(c == NCH - 1),
                    )
            cb_ps = pss.tile([BLOC, 1], F32, name="cb_ps", tag="pss")
            for c in range(NCH):
                nc.tensor.matmul(
                    cb_ps,
                    QT_sb[:, c, :].bitcast(F32),
                    bk8_sb[:, c : c + 1],
                    start=(c == 0),
                    stop=(c == NCH - 1),
                )
            qt4_sb = small.tile([BLOC, TP1], F32R, name="qt4_sb", tag="qt4_sb")
            for h in range(2):
                nc.scalar.mul(
                    qt4_sb[:, h * 512 : (h + 1) * 512],
                    psq2[:, h * 512 : (h + 1) * 512],
                    SCALE,
                )
            nc.scalar.mul(qt4_sb[:, T:TP1], cb_ps, SCALE)

            # ---- broadcast a batch's qt row to 128 partitions on PE;
            #      col T carries the scaled cb bias.  Emitted lazily: qb[0]
            #      before the stream, qb[b+1] during batch b's stream, so
            #      the PE<->scalar sem ping-pong (~3 us/batch) overlaps the
            #      stream instead of delaying the first DVE chunk ----
            qb_all = small.tile([P, BLOC, TP1], BF16, name="qb_all", tag="qb_all")

            def _emit_qb(b):
                for h in range(2):
                    qps = psq.tile([P, 512], F32, name="qps", tag="psq")
                    nc.tensor.matmul(
                        qps,
                        sel_sb[:, b * P : (b + 1) * P],
                        qt4_sb[:, h * 512 : (h + 1) * 512],
                        start=True,
                        stop=True,
                    )
                    nc.scalar.copy(qb_all[:, b, h * 512 : (h + 1) * 512], qps)
                qpc = pss.tile([P, 1], F32, name="qpc", tag="pss")
                nc.tensor.matmul(
                    qpc,
                    sel_sb[:, b * P : (b + 1) * P].bitcast(F32),
                    qt4_sb[:, T:TP1].bitcast(F32),
                    start=True,
                    stop=True,
                )
                nc.scalar.copy(qb_all[:, b, T:TP1], qpc)

            qb_t = [qb_all[:, b, :] for b in range(BLOC)]
            _emit_qb(0)

            # ---- main stream: e = (rt . qt)+cb ; ww += e.T-weighted rows ----
            for b in range(BLOC):
                ps_ww = psA.tile([1, T], F32, name="ps_ww", tag="psA")
                for ti in range(NTI):
                    if ti == 1 and b + 1 < BLOC:
                        _emit_qb(b + 1)
                    n = b * NTI + ti
                    k = n % RTBUFS
                    e_raw = ep.tile([P, LTILE], F32, name="e_raw", tag="e_raw")
                    e_t = ep.tile([P, LTILE], BF16, name="e_t", tag="e_t")
                    for i in range(LTILE):
                        scr = scrp.tile([P, T], F32, name="scr", tag="scr")
                        nc.vector.scalar_tensor_tensor(
                            out=scr,
                            in0=rtb[k][:, i, :],
                            scalar=1.0,
                            in1=qb_t[b][:, 0:T],
                            op0=OP.bypass,
                            op1=OP.mult,
                            accum_out=e_raw[:, i : i + 1],
                        )
                        nc.scalar.activation(
                            e_t[:, i : i + 1],
                            e_raw[:, i : i + 1],
                            FT.Identity,
                            bias=qb_t[b][:, T : T + 1],
                        )
                        for h in range(2):
                            nc.tensor.matmul(
                                ps_ww[0:1, h * 512 : (h + 1) * 512],
                                e_t[:, i : i + 1],
                                rtb[k][:, i, h * 512 : (h + 1) * 512],
                                start=(ti == 0 and i == 0),
                                stop=(ti == NTI - 1 and i == LTILE - 1),
                            )
                    if n + RTBUFS < NTILES:
                        _issue_rt(n + RTBUFS)
                ww_sb = wwp.tile([1, T], F32, name="ww_sb", tag="ww_sb")
                nc.scalar.copy(ww_sb[0:1, 0:512], ps_ww[0:1, 0:512])
                nc.scalar.copy(ww_sb[0:1, 512:1024], ps_ww[0:1, 512:1024])
                nc.scalar.dma_start(out=out[b : b + 1, :], in_=ww_sb)

    nc.compile()
    return nc


_NC = None


def _get_nc():
    global _NC
    if _NC is None:
        _NC = _build()
    return _NC


def _in_maps(submitter_emb, reviewer_emb, Wq, bq, Wk, bk):
    submitter_emb = np.ascontiguousarray(submitter_emb, dtype=np.float32)
    reviewer_emb = np.ascontiguousarray(reviewer_emb, dtype=np.float32)
    Wq = np.asarray(Wq, dtype=np.float32)
    Wk = np.asarray(Wk, dtype=np.float32)
    bq = np.asarray(bq, dtype=np.float32)
    bk = np.asarray(bk, dtype=np.float32)

    # wqtf[p, j, a] = Wq[a, 128j + p]
    wqtf = np.ascontiguousarray(Wq.T.reshape(NCH, P, A).transpose(1, 0, 2))
    # wkf[p, c, t] = Wk[128c + p, t]
    wkf = np.ascontiguousarray(Wk.reshape(NCH, P, T).transpose(1, 0, 2))
    bqr = np.ascontiguousarray(bq.reshape(1, A))
    bk8 = np.ascontiguousarray(bk.reshape(NCH, P).T)
    ones = np.ones((1, BLOC), dtype=np.float32)
    sel4 = np.zeros((BLOC, BLOC * P), dtype=np.float32)
    for b_ in range(BLOC):
        sel4[b_, b_ * P : (b_ + 1) * P] = 1.0

    in_maps = []
    for core in range(NCORES):
        lo, hi = core * BLOC, (core + 1) * BLOC
        subt = np.ascontiguousarray(
            submitter_emb[lo:hi].T.reshape(NCH, P, BLOC).transpose(1, 0, 2)
        )
        in_maps.append(
            {
                "rev": reviewer_emb[lo:hi],
                "subt": subt,
                "wqtf": wqtf,
                "wkf": wkf,
                "bqr": bqr,
                "bk8": bk8,
                "ones": ones,
                "sel4": sel4,
            }
        )
    return in_maps


def kernel(
    submitter_emb: np.ndarray,
    reviewer_emb: np.ndarray,
    Wq: np.ndarray,
    bq: np.ndarray,
    Wk: np.ndarray,
    bk: np.ndarray,
) -> np.ndarray:
    nc = _get_nc()
    in_maps = _in_maps(submitter_emb, reviewer_emb, Wq, bq, Wk, bk)
    res = run_bass_kernel_spmd(nc, in_maps, core_ids=list(range(NCORES)))
    return np.concatenate([res.results[c]["out"] for c in range(NCORES)], axis=0)

